# revision 6
# baseline (speedup 1.0000x reference)
"""Trainium2 Bass kernel for nn_DLI_loss_full.

Key algebraic fact: logits[b,j,k] = hw[b,j] + xw[b,k] and the loss is
sum(lse - tgt) over valid groups, so the hw[b,j] term (the whole LSTM
path) cancels exactly:

    per_group[b,j] = log(sum_{k=j+1}^{len_b-1} exp(xw[b,k])) - xw[b,j+1]
    loss = sum(per_group) / sum_b(len_b - 1)

with xw = encoder_output @ w_fc[HID:].  The kernel therefore only
streams encoder_output once (memory-bound), computes xw via
multiply+256-wide reductions, then gets every suffix log-sum-exp with
one hardware suffix-sum scan per 48-wide chunk plus a cross-chunk
combine done as a tiny 128x128 matmul.

Per-core layout: 16 batches x 8 chunks of 48 timesteps = 128 SBUF
partitions, each partition's encoder slice contiguous in DRAM.  The
encoder stream rides the scalar HWDGE ring alone (~340-420 B/ns
sustained); consts ride the sync ring.

Engine plan (v4, measured op costs): DVE is the conveyor - it runs
multiply (1.75us) or reduce (1.75us) back to back from the moment
piece 0 lands; gpsimd shadows it with the 4 middle multiplies (3.4us
each - its 2-input port-bound floor).  Three-engine concurrency (ACT
reduces) was tried and SLOWED everything ~45% via SBUF contention, so
ACT only does the exp/ln tail.  The mask arrives pre-cast fp32 and
pre-multiplied by the chunk mask from the host, so gpsimd runs nothing
but the 4 multiplies.  The final [128,2] result is collapsed to [2,1]
by a PE matmul against a ones column so the output DMA is 2
descriptors instead of 128.
"""

from contextlib import ExitStack

import numpy as np

import concourse.bacc as bacc
import concourse.mybir as mybir
import concourse.tile as tile
from concourse import bass_utils

B, T, D, HID = 128, 384, 256, 256
NCORES = 8
BS = B // NCORES            # 16 batches per core
CH = 8                      # chunks per sequence
L = T // CH                 # 48 timesteps per chunk
P = BS * CH                 # 128 partitions
NP = 8                      # DMA/compute pieces along the free axis
LP = L // NP                # 6 timesteps per piece
F32 = mybir.dt.float32
EPS = 1e-30                 # keeps ln() finite on fully-masked tails
C2W = 2 * L + P + 1         # mkf | wmh | um | ones

# pieces whose multiply runs on gpsimd; DVE multiplies 0,1,6,7 itself
GP_PIECES = (2, 3, 4, 5)

_cache = {}


def _build_nc():
    nc = bacc.Bacc(
        "TRN2", target_bir_lowering=False, debug=False, num_devices=NCORES
    )
    x = nc.dram_tensor("x", [BS, T, D], F32, kind="ExternalInput").ap()
    wt = nc.dram_tensor("wt", [P, D], F32, kind="ExternalInput").ap()
    c2 = nc.dram_tensor("c2", [P, C2W], F32, kind="ExternalInput").ap()
    out = nc.dram_tensor("out", [2, 1], F32, kind="ExternalOutput").ap()

    add = mybir.AluOpType.add
    mult = mybir.AluOpType.mult
    bypass = mybir.AluOpType.bypass
    AX = mybir.AxisListType.X
    ACT = mybir.ActivationFunctionType

    with tile.TileContext(nc) as tc, ExitStack() as ctx:
        sp = ctx.enter_context(tc.tile_pool(name="small", bufs=1))
        xp = ctx.enter_context(tc.tile_pool(name="xp", bufs=NP))
        rp = ctx.enter_context(tc.tile_pool(name="prod", bufs=4))
        pp = ctx.enter_context(tc.tile_pool(name="psum", bufs=2, space="PSUM"))

        # x-piece loads first, all on the scalar HWDGE queue
        x_p = x.rearrange("b (c n l) d -> (b c) n (l d)", c=CH, n=NP)
        xts = []
        for i in range(NP):
            xt = xp.tile([P, LP * D], F32, tag="x")
            nc.scalar.dma_start(xt[:], x_p[:, i, :])
            xts.append(xt)

        # consts on the sync ring: wt alone first (it gates wrep), then
        # the rest packed into one block
        w_sb = sp.tile([P, D], F32)
        nc.sync.dma_start(w_sb[:], wt)
        c2_sb = sp.tile([P, C2W], F32)
        nc.sync.dma_start(c2_sb[:], c2)
        mf = c2_sb[:, 0:L]                      # fp32 mask (host-cast)
        wm = c2_sb[:, L:2 * L]                  # mask * chunk-mask (host)
        u_sb = c2_sb[:, 2 * L:2 * L + P]        # cross-chunk combine
        ones = c2_sb[:, 2 * L + P:2 * L + P + 1]

        # warm the Exp table while DMA streams (reads w, the earliest
        # const); the Ln table load is pinned after the real exp below
        warm = sp.tile([P, 1], F32)
        nc.scalar.activation(warm[:], w_sb[:, 0:1], ACT.Exp)

        # replicate w LP times on-chip so the multiplies read a plain
        # contiguous operand (0-stride broadcast halves DVE rate)
        wrep = sp.tile([P, LP * D], F32)
        nc.vector.tensor_copy(wrep[:, 0:D], w_sb[:])
        nc.vector.tensor_copy(wrep[:, D:2 * D], wrep[:, 0:D])
        nc.vector.tensor_copy(wrep[:, 2 * D:4 * D], wrep[:, 0:2 * D])
        nc.vector.tensor_copy(wrep[:, 4 * D:6 * D], wrep[:, 2 * D:4 * D])

        # products: gpsimd multiplies the middle pieces
        pts = [None] * NP
        for i in GP_PIECES:
            pts[i] = rp.tile([P, LP * D], F32, tag="prod", name=f"pt{i}")
            nc.gpsimd.tensor_tensor(pts[i][:], xts[i][:], wrep[:], mult)

        # xw[p, t] = sum_d x[p, t, d] * w[d]: DVE multiplies pieces
        # 0,1,6,7 and reduces everything, enqueued in expected
        # data-readiness order (engine queues run in-order)
        xw = sp.tile([P, L], F32)

        def _reduce(i):
            p3 = pts[i][:].rearrange("p (l d) -> p l d", d=D)
            nc.vector.tensor_reduce(
                xw[:, i * LP:(i + 1) * LP], p3, axis=AX, op=add
            )

        def _vmult(i):
            pts[i] = rp.tile([P, LP * D], F32, tag="prod", name=f"pt{i}")
            nc.vector.tensor_tensor(pts[i][:], xts[i][:], wrep[:], mult)

        _vmult(0)
        _reduce(0)
        _vmult(1)
        _reduce(1)
        _reduce(2)
        _reduce(3)
        _vmult(6)
        _reduce(4)
        _vmult(7)
        _reduce(5)
        _reduce(6)
        _reduce(7)

        # masked exp, chunk totals, cross-chunk exclusive suffix via matmul
        em = sp.tile([P, L], F32)
        nc.scalar.activation(em[:], xw[:], ACT.Exp)
        # reads em -> cannot be hoisted before the exp; triggers the Ln
        # table load here so it overlaps the DVE tail below
        lnwarm = sp.tile([P, 1], F32)
        nc.scalar.activation(lnwarm[:], em[:, 0:1], ACT.Ln)
        nc.vector.tensor_mul(em[:], em[:], mf)
        tot = sp.tile([P, 1], F32)
        nc.vector.tensor_reduce(tot[:], em[:], axis=AX, op=add)
        aps = pp.tile([P, 1], F32, tag="mm")
        nc.tensor.matmul(aps[:], u_sb, tot[:], start=True, stop=True)
        a_sb = sp.tile([P, 1], F32)
        # + EPS seeds every suffix sum, keeping ln() finite on
        # fully-masked tails
        nc.vector.tensor_scalar_add(a_sb[:], aps[:], EPS)

        # within-chunk suffix sums, seeded with the later-chunk total
        ss = sp.tile([P, L], F32)
        nc.vector.tensor_tensor_scan(
            ss[:][:, ::-1], em[:][:, ::-1], em[:][:, ::-1],
            initial=a_sb[:], op0=add, op1=bypass,
        )
        lt = sp.tile([P, L], F32)
        nc.scalar.activation(lt[:], ss[:], ACT.Ln)

        # loss terms: per-partition sum of (ln(suffix) - xw) over valid
        # groups, and the valid-group count; then collapse across
        # partitions with a tiny matmul so the output is [2,1]
        diff = sp.tile([P, L], F32)
        nc.vector.tensor_sub(diff[:], lt[:], xw[:])
        res = sp.tile([P, 2], F32)
        nc.vector.scalar_tensor_tensor(
            out=diff[:], in0=diff[:], scalar=1.0, in1=wm,
            op0=bypass, op1=mult, accum_out=res[:, 0:1],
        )
        nc.vector.tensor_reduce(res[:, 1:2], mf, axis=AX, op=add)
        aps2 = pp.tile([2, 1], F32, tag="mm2")
        nc.tensor.matmul(aps2[:], res[:], ones, start=True, stop=True)
        res2 = sp.tile([2, 1], F32)
        nc.vector.tensor_copy(res2[:], aps2[:])
        nc.sync.dma_start(out, res2[:])

    nc.compile()
    return nc


def _host_consts():
    w_idx = np.arange(P)
    um = (
        (w_idx[:, None] // CH == w_idx[None, :] // CH)
        & (w_idx[:, None] % CH > w_idx[None, :] % CH)
    ).astype(np.float32)
    cm = np.ones((P, L), np.float32)
    cm[w_idx % CH == 0, 0] = 0.0
    return um, cm


def _core_c2(mask_core, um, cm):
    """Pack mkf | wmh | um | ones into one [P, C2W] block."""
    mkf = np.asarray(mask_core).reshape(P, L).astype(np.float32)
    c2 = np.empty((P, C2W), np.float32)
    c2[:, 0:L] = mkf
    c2[:, L:2 * L] = mkf * cm
    c2[:, 2 * L:2 * L + P] = um
    c2[:, 2 * L + P] = 1.0
    return c2


def kernel(**inputs) -> np.ndarray:
    enc = np.ascontiguousarray(np.asarray(inputs["encoder_output"], np.float32))
    mask = np.asarray(inputs["mask"])
    w_fc = np.asarray(inputs["w_fc"], np.float32)

    if "nc" not in _cache:
        _cache["nc"] = _build_nc()
    nc = _cache["nc"]

    wt = np.ascontiguousarray(np.broadcast_to(w_fc[HID:], (P, D)), np.float32)
    um, cm = _host_consts()
    in_maps = [
        {
            "x": enc[c * BS:(c + 1) * BS],
            "wt": wt,
            "c2": _core_c2(mask[c * BS:(c + 1) * BS], um, cm),
        }
        for c in range(NCORES)
    ]
    res = bass_utils.run_bass_kernel_spmd(
        nc, in_maps, core_ids=list(range(NCORES))
    )
    o = np.stack([r["out"] for r in res.results]).astype(np.float64)
    num = o[:, 0, 0].sum()
    den = o[:, 1, 0].sum() - B
    return np.asarray(num / den, dtype=np.float32)


# revision 7
# speedup vs baseline: 1.0772x; 1.0772x over previous
"""Trainium2 Bass kernel for nn_DLI_loss_full.

Key algebraic fact: logits[b,j,k] = hw[b,j] + xw[b,k] and the loss is
sum(lse - tgt) over valid groups, so the hw[b,j] term (the whole LSTM
path) cancels exactly:

    per_group[b,j] = log(sum_{k=j+1}^{len_b-1} exp(xw[b,k])) - xw[b,j+1]
    loss = sum(per_group) / sum_b(len_b - 1)

with xw = encoder_output @ w_fc[HID:].  The kernel therefore only
streams encoder_output once (memory-bound), computes xw via
multiply+256-wide reductions, then gets every suffix log-sum-exp with
one hardware suffix-sum scan per 48-wide chunk plus a cross-chunk
combine done as a tiny 128x128 matmul.

Per-core layout: 16 batches x 8 chunks of 48 timesteps = 128 SBUF
partitions, each partition's encoder slice contiguous in DRAM.  The
encoder stream rides the scalar HWDGE ring alone (~340-420 B/ns
sustained); consts ride the sync ring.

Engine plan (v4, measured op costs): DVE is the conveyor - it runs
multiply (1.75us) or reduce (1.75us) back to back from the moment
piece 0 lands; gpsimd shadows it with the 4 middle multiplies (3.4us
each - its 2-input port-bound floor).  Three-engine concurrency (ACT
reduces) was tried and SLOWED everything ~45% via SBUF contention, so
ACT only does the exp/ln tail.  The mask arrives pre-cast fp32 and
pre-multiplied by the chunk mask from the host, so gpsimd runs nothing
but the 4 multiplies.  The final [128,2] result is collapsed to [2,1]
by a PE matmul against a ones column so the output DMA is 2
descriptors instead of 128.
"""

from contextlib import ExitStack

import numpy as np

import concourse.bacc as bacc
import concourse.mybir as mybir
import concourse.tile as tile
from concourse import bass_utils

B, T, D, HID = 128, 384, 256, 256
NCORES = 8
BS = B // NCORES            # 16 batches per core
CH = 8                      # chunks per sequence
L = T // CH                 # 48 timesteps per chunk
P = BS * CH                 # 128 partitions
NP = 8                      # DMA/compute pieces along the free axis
LP = L // NP                # 6 timesteps per piece
F32 = mybir.dt.float32
EPS = 1e-30                 # keeps ln() finite on fully-masked tails
C2W = 2 * L + P + 1         # mkf | wmh | um | ones

# pieces whose multiply runs on gpsimd; DVE multiplies 0,1,6,7 itself
GP_PIECES = (2, 3, 4, 5)

_cache = {}


def _build_nc():
    nc = bacc.Bacc(
        "TRN2", target_bir_lowering=False, debug=False, num_devices=NCORES
    )
    x = nc.dram_tensor("x", [BS, T, D], F32, kind="ExternalInput").ap()
    wt = nc.dram_tensor("wt", [P, D], F32, kind="ExternalInput").ap()
    c2 = nc.dram_tensor("c2", [P, C2W], F32, kind="ExternalInput").ap()
    out = nc.dram_tensor("out", [1, 1], F32, kind="ExternalOutput").ap()

    add = mybir.AluOpType.add
    mult = mybir.AluOpType.mult
    bypass = mybir.AluOpType.bypass
    AX = mybir.AxisListType.X
    ACT = mybir.ActivationFunctionType

    with tile.TileContext(nc) as tc, ExitStack() as ctx:
        sp = ctx.enter_context(tc.tile_pool(name="small", bufs=1))
        xp = ctx.enter_context(tc.tile_pool(name="xp", bufs=NP))
        rp = ctx.enter_context(tc.tile_pool(name="prod", bufs=4))
        pp = ctx.enter_context(tc.tile_pool(name="psum", bufs=2, space="PSUM"))

        # x-piece loads first, all on the scalar HWDGE queue
        x_p = x.rearrange("b (c n l) d -> (b c) n (l d)", c=CH, n=NP)
        xts = []
        for i in range(NP):
            xt = xp.tile([P, LP * D], F32, tag="x")
            nc.scalar.dma_start(xt[:], x_p[:, i, :])
            xts.append(xt)

        # consts on the sync ring: wt alone first (it gates wrep), then
        # the rest packed into one block
        w_sb = sp.tile([P, D], F32)
        nc.sync.dma_start(w_sb[:], wt)
        c2_sb = sp.tile([P, C2W], F32)
        nc.sync.dma_start(c2_sb[:], c2)
        mf = c2_sb[:, 0:L]                      # fp32 mask (host-cast)
        wm = c2_sb[:, L:2 * L]                  # mask * chunk-mask (host)
        u_sb = c2_sb[:, 2 * L:2 * L + P]        # cross-chunk combine
        ones = c2_sb[:, 2 * L + P:2 * L + P + 1]

        # warm the Exp table while DMA streams (reads w, the earliest
        # const); the Ln table load is pinned after the real exp below
        warm = sp.tile([P, 1], F32)
        nc.scalar.activation(warm[:], w_sb[:, 0:1], ACT.Exp)

        # replicate w LP times on-chip so the multiplies read a plain
        # contiguous operand (0-stride broadcast halves DVE rate)
        wrep = sp.tile([P, LP * D], F32)
        nc.vector.tensor_copy(wrep[:, 0:D], w_sb[:])
        nc.vector.tensor_copy(wrep[:, D:2 * D], wrep[:, 0:D])
        nc.vector.tensor_copy(wrep[:, 2 * D:4 * D], wrep[:, 0:2 * D])
        nc.vector.tensor_copy(wrep[:, 4 * D:6 * D], wrep[:, 2 * D:4 * D])

        # products: gpsimd multiplies the middle pieces
        pts = [None] * NP
        for i in GP_PIECES:
            pts[i] = rp.tile([P, LP * D], F32, tag="prod", name=f"pt{i}")
            nc.gpsimd.tensor_tensor(pts[i][:], xts[i][:], wrep[:], mult)

        # xw[p, t] = sum_d x[p, t, d] * w[d]: DVE multiplies pieces
        # 0,1,6,7 and reduces everything, enqueued in expected
        # data-readiness order (engine queues run in-order)
        xw = sp.tile([P, L], F32)

        def _reduce(i):
            p3 = pts[i][:].rearrange("p (l d) -> p l d", d=D)
            nc.vector.tensor_reduce(
                xw[:, i * LP:(i + 1) * LP], p3, axis=AX, op=add
            )

        def _vmult(i):
            pts[i] = rp.tile([P, LP * D], F32, tag="prod", name=f"pt{i}")
            nc.vector.tensor_tensor(pts[i][:], xts[i][:], wrep[:], mult)

        _vmult(0)
        _reduce(0)
        _vmult(1)
        _reduce(1)
        _reduce(2)
        _reduce(3)
        _vmult(6)
        _reduce(4)
        _vmult(7)
        _reduce(5)
        _reduce(6)
        _reduce(7)

        # masked exp, chunk totals, cross-chunk exclusive suffix via matmul
        em = sp.tile([P, L], F32)
        nc.scalar.activation(em[:], xw[:], ACT.Exp)
        # reads em -> cannot be hoisted before the exp; triggers the Ln
        # table load here so it overlaps the DVE tail below
        lnwarm = sp.tile([P, 1], F32)
        nc.scalar.activation(lnwarm[:], em[:, 0:1], ACT.Ln)
        em2 = sp.tile([P, L], F32)
        nc.vector.tensor_mul(em2[:], em[:], mf)
        tot = sp.tile([P, 1], F32)
        nc.vector.tensor_reduce(tot[:], em2[:], axis=AX, op=add)
        aps = pp.tile([P, 1], F32, tag="mm")
        nc.tensor.matmul(aps[:], u_sb, tot[:], start=True, stop=True)
        a_sb = sp.tile([P, 1], F32)
        # + EPS seeds every suffix sum, keeping ln() finite on
        # fully-masked tails
        nc.vector.tensor_scalar_add(a_sb[:], aps[:], EPS)

        # within-chunk suffix sums, seeded with the later-chunk total
        ss = sp.tile([P, L], F32)
        nc.vector.tensor_tensor_scan(
            ss[:][:, ::-1], em2[:][:, ::-1], em2[:][:, ::-1],
            initial=a_sb[:], op0=add, op1=bypass,
        )
        lt = sp.tile([P, L], F32)
        nc.scalar.activation(lt[:], ss[:], ACT.Ln)

        # loss terms: per-partition sum of (ln(suffix) - xw) over valid
        # groups, and the valid-group count; then collapse across
        # partitions with a tiny matmul so the output is [2,1]
        diff = sp.tile([P, L], F32)
        nc.vector.tensor_sub(diff[:], lt[:], xw[:])
        res = sp.tile([P, 1], F32)
        nc.vector.scalar_tensor_tensor(
            out=diff[:], in0=diff[:], scalar=1.0, in1=wm,
            op0=bypass, op1=mult, accum_out=res[:, 0:1],
        )
        aps2 = pp.tile([1, 1], F32, tag="mm2")
        nc.tensor.matmul(aps2[:], res[:], ones, start=True, stop=True)
        res2 = sp.tile([1, 1], F32)
        nc.vector.tensor_copy(res2[:], aps2[:])
        nc.sync.dma_start(out, res2[:])

    nc.compile()
    return nc


def _host_consts():
    w_idx = np.arange(P)
    um = (
        (w_idx[:, None] // CH == w_idx[None, :] // CH)
        & (w_idx[:, None] % CH > w_idx[None, :] % CH)
    ).astype(np.float32)
    cm = np.ones((P, L), np.float32)
    cm[w_idx % CH == 0, 0] = 0.0
    return um, cm


def _core_c2(mask_core, um, cm):
    """Pack mkf | wmh | um | ones into one [P, C2W] block."""
    mkf = np.asarray(mask_core).reshape(P, L).astype(np.float32)
    c2 = np.empty((P, C2W), np.float32)
    c2[:, 0:L] = mkf
    c2[:, L:2 * L] = mkf * cm
    c2[:, 2 * L:2 * L + P] = um
    c2[:, 2 * L + P] = 1.0
    return c2


def kernel(**inputs) -> np.ndarray:
    enc = np.ascontiguousarray(np.asarray(inputs["encoder_output"], np.float32))
    mask = np.asarray(inputs["mask"])
    w_fc = np.asarray(inputs["w_fc"], np.float32)

    if "nc" not in _cache:
        _cache["nc"] = _build_nc()
    nc = _cache["nc"]

    wt = np.ascontiguousarray(np.broadcast_to(w_fc[HID:], (P, D)), np.float32)
    um, cm = _host_consts()
    in_maps = [
        {
            "x": enc[c * BS:(c + 1) * BS],
            "wt": wt,
            "c2": _core_c2(mask[c * BS:(c + 1) * BS], um, cm),
        }
        for c in range(NCORES)
    ]
    res = bass_utils.run_bass_kernel_spmd(
        nc, in_maps, core_ids=list(range(NCORES))
    )
    o = np.stack([r["out"] for r in res.results]).astype(np.float64)
    num = o[:, 0, 0].sum()
    den = float(np.asarray(mask, np.int64).sum() - B)
    return np.asarray(num / den, dtype=np.float32)


# revision 8
# speedup vs baseline: 1.0862x; 1.0083x over previous
"""Trainium2 Bass kernel for nn_DLI_loss_full.

Key algebraic fact: logits[b,j,k] = hw[b,j] + xw[b,k] and the loss is
sum(lse - tgt) over valid groups, so the hw[b,j] term (the whole LSTM
path) cancels exactly:

    per_group[b,j] = log(sum_{k=j+1}^{len_b-1} exp(xw[b,k])) - xw[b,j+1]
    loss = sum(per_group) / sum_b(len_b - 1)

with xw = encoder_output @ w_fc[HID:].  The kernel therefore only
streams encoder_output once (memory-bound), computes xw via
multiply+256-wide reductions, then gets every suffix log-sum-exp with
one hardware suffix-sum scan per 48-wide chunk plus a cross-chunk
combine done as a tiny 128x128 matmul.

Per-core layout: 16 batches x 8 chunks of 48 timesteps = 128 SBUF
partitions, each partition's encoder slice contiguous in DRAM.  The
encoder stream rides the scalar HWDGE ring alone (~340-420 B/ns
sustained); consts ride the sync ring.

Engine plan (v4, measured op costs): DVE is the conveyor - it runs
multiply (1.75us) or reduce (1.75us) back to back from the moment
piece 0 lands; gpsimd shadows it with the 4 middle multiplies (3.4us
each - its 2-input port-bound floor).  Three-engine concurrency (ACT
reduces) was tried and SLOWED everything ~45% via SBUF contention, so
ACT only does the exp/ln tail.  The mask arrives pre-cast fp32 and
pre-multiplied by the chunk mask from the host, so gpsimd runs nothing
but the 4 multiplies.  The final [128,2] result is collapsed to [2,1]
by a PE matmul against a ones column so the output DMA is 2
descriptors instead of 128.
"""

from contextlib import ExitStack

import numpy as np

import concourse.bacc as bacc
import concourse.mybir as mybir
import concourse.tile as tile
from concourse import bass_utils

B, T, D, HID = 128, 384, 256, 256
NCORES = 8
BS = B // NCORES            # 16 batches per core
CH = 8                      # chunks per sequence
L = T // CH                 # 48 timesteps per chunk
P = BS * CH                 # 128 partitions
NP = 8                      # DMA/compute pieces along the free axis
LP = L // NP                # 6 timesteps per piece
F32 = mybir.dt.float32
EPS = 1e-30                 # keeps ln() finite on fully-masked tails
C2W = 2 * L + P + 1         # mkf | wmh | um | ones

# pieces whose multiply runs on gpsimd; DVE multiplies 0,1,6,7 itself
GP_PIECES = (2, 3, 4, 5)

_cache = {}


def _build_nc():
    nc = bacc.Bacc(
        "TRN2", target_bir_lowering=False, debug=False, num_devices=NCORES
    )
    x = nc.dram_tensor("x", [BS, T, D], F32, kind="ExternalInput").ap()
    wt = nc.dram_tensor("wt", [P, D], F32, kind="ExternalInput").ap()
    c2 = nc.dram_tensor("c2", [P, C2W], F32, kind="ExternalInput").ap()
    out = nc.dram_tensor("out", [1, 1], F32, kind="ExternalOutput").ap()

    add = mybir.AluOpType.add
    mult = mybir.AluOpType.mult
    bypass = mybir.AluOpType.bypass
    AX = mybir.AxisListType.X
    ACT = mybir.ActivationFunctionType

    with tile.TileContext(nc) as tc, ExitStack() as ctx:
        sp = ctx.enter_context(tc.tile_pool(name="small", bufs=1))
        xp = ctx.enter_context(tc.tile_pool(name="xp", bufs=NP))
        rp = ctx.enter_context(tc.tile_pool(name="prod", bufs=NP))
        pp = ctx.enter_context(tc.tile_pool(name="psum", bufs=2, space="PSUM"))

        # x-piece loads first, all on the scalar HWDGE queue
        x_p = x.rearrange("b (c n l) d -> (b c) n (l d)", c=CH, n=NP)
        xts = []
        for i in range(NP):
            xt = xp.tile([P, LP * D], F32, tag="x")
            nc.scalar.dma_start(xt[:], x_p[:, i, :])
            xts.append(xt)

        # consts on the sync ring: wt alone first (it gates wrep), then
        # the rest packed into one block
        w_sb = sp.tile([P, D], F32)
        nc.sync.dma_start(w_sb[:], wt)
        c2_sb = sp.tile([P, C2W], F32)
        nc.sync.dma_start(c2_sb[:], c2)
        mf = c2_sb[:, 0:L]                      # fp32 mask (host-cast)
        wm = c2_sb[:, L:2 * L]                  # mask * chunk-mask (host)
        u_sb = c2_sb[:, 2 * L:2 * L + P]        # cross-chunk combine
        ones = c2_sb[:, 2 * L + P:2 * L + P + 1]

        # warm the Exp table while DMA streams (reads w, the earliest
        # const); the Ln table load is pinned after the real exp below
        warm = sp.tile([P, 1], F32)
        nc.scalar.activation(warm[:], w_sb[:, 0:1], ACT.Exp)

        # replicate w LP times on-chip so the multiplies read a plain
        # contiguous operand (0-stride broadcast halves DVE rate)
        wrep = sp.tile([P, LP * D], F32)
        nc.vector.tensor_copy(wrep[:, 0:D], w_sb[:])
        nc.vector.tensor_copy(wrep[:, D:2 * D], wrep[:, 0:D])
        nc.vector.tensor_copy(wrep[:, 2 * D:4 * D], wrep[:, 0:2 * D])
        nc.vector.tensor_copy(wrep[:, 4 * D:6 * D], wrep[:, 2 * D:4 * D])

        # products: gpsimd multiplies the middle pieces
        pts = [None] * NP
        for i in GP_PIECES:
            pts[i] = rp.tile([P, LP * D], F32, tag="prod", name=f"pt{i}")
            nc.gpsimd.tensor_tensor(pts[i][:], xts[i][:], wrep[:], mult)

        # xw[p, t] = sum_d x[p, t, d] * w[d]: DVE multiplies pieces
        # 0,1,6,7 and reduces everything, enqueued in expected
        # data-readiness order (engine queues run in-order)
        xw = sp.tile([P, L], F32)

        def _reduce(i):
            p3 = pts[i][:].rearrange("p (l d) -> p l d", d=D)
            nc.vector.tensor_reduce(
                xw[:, i * LP:(i + 1) * LP], p3, axis=AX, op=add
            )

        def _vmult(i):
            pts[i] = rp.tile([P, LP * D], F32, tag="prod", name=f"pt{i}")
            nc.vector.tensor_tensor(pts[i][:], xts[i][:], wrep[:], mult)

        _vmult(0)
        _reduce(0)
        _vmult(1)
        _reduce(1)
        _reduce(2)
        _reduce(3)
        _vmult(6)
        _reduce(4)
        _vmult(7)
        _reduce(5)
        _reduce(6)
        _reduce(7)

        # masked exp, chunk totals, cross-chunk exclusive suffix via matmul
        em = sp.tile([P, L], F32)
        nc.scalar.activation(em[:], xw[:], ACT.Exp)
        # reads em -> cannot be hoisted before the exp; triggers the Ln
        # table load here so it overlaps the DVE tail below
        lnwarm = sp.tile([P, 1], F32)
        nc.scalar.activation(lnwarm[:], em[:, 0:1], ACT.Ln)
        em2 = sp.tile([P, L], F32)
        nc.vector.tensor_mul(em2[:], em[:], mf)
        tot = sp.tile([P, 1], F32)
        nc.vector.tensor_reduce(tot[:], em2[:], axis=AX, op=add)
        aps = pp.tile([P, 1], F32, tag="mm")
        nc.tensor.matmul(aps[:], u_sb, tot[:], start=True, stop=True)
        a_sb = sp.tile([P, 1], F32)
        # + EPS seeds every suffix sum, keeping ln() finite on
        # fully-masked tails
        nc.vector.tensor_scalar_add(a_sb[:], aps[:], EPS)

        # within-chunk suffix sums, seeded with the later-chunk total
        ss = sp.tile([P, L], F32)
        nc.vector.tensor_tensor_scan(
            ss[:][:, ::-1], em2[:][:, ::-1], em2[:][:, ::-1],
            initial=a_sb[:], op0=add, op1=bypass,
        )
        lt = sp.tile([P, L], F32)
        nc.scalar.activation(lt[:], ss[:], ACT.Ln)

        # loss terms: per-partition sum of (ln(suffix) - xw) over valid
        # groups, and the valid-group count; then collapse across
        # partitions with a tiny matmul so the output is [2,1]
        diff = sp.tile([P, L], F32)
        nc.vector.tensor_sub(diff[:], lt[:], xw[:])
        res = sp.tile([P, 1], F32)
        nc.vector.scalar_tensor_tensor(
            out=diff[:], in0=diff[:], scalar=1.0, in1=wm,
            op0=bypass, op1=mult, accum_out=res[:, 0:1],
        )
        aps2 = pp.tile([1, 1], F32, tag="mm2")
        nc.tensor.matmul(aps2[:], res[:], ones, start=True, stop=True)
        res2 = sp.tile([1, 1], F32)
        nc.vector.tensor_copy(res2[:], aps2[:])
        nc.sync.dma_start(out, res2[:])

    nc.compile()
    return nc


def _host_consts():
    w_idx = np.arange(P)
    um = (
        (w_idx[:, None] // CH == w_idx[None, :] // CH)
        & (w_idx[:, None] % CH > w_idx[None, :] % CH)
    ).astype(np.float32)
    cm = np.ones((P, L), np.float32)
    cm[w_idx % CH == 0, 0] = 0.0
    return um, cm


def _core_c2(mask_core, um, cm):
    """Pack mkf | wmh | um | ones into one [P, C2W] block."""
    mkf = np.asarray(mask_core).reshape(P, L).astype(np.float32)
    c2 = np.empty((P, C2W), np.float32)
    c2[:, 0:L] = mkf
    c2[:, L:2 * L] = mkf * cm
    c2[:, 2 * L:2 * L + P] = um
    c2[:, 2 * L + P] = 1.0
    return c2


def kernel(**inputs) -> np.ndarray:
    enc = np.ascontiguousarray(np.asarray(inputs["encoder_output"], np.float32))
    mask = np.asarray(inputs["mask"])
    w_fc = np.asarray(inputs["w_fc"], np.float32)

    if "nc" not in _cache:
        _cache["nc"] = _build_nc()
    nc = _cache["nc"]

    wt = np.ascontiguousarray(np.broadcast_to(w_fc[HID:], (P, D)), np.float32)
    um, cm = _host_consts()
    in_maps = [
        {
            "x": enc[c * BS:(c + 1) * BS],
            "wt": wt,
            "c2": _core_c2(mask[c * BS:(c + 1) * BS], um, cm),
        }
        for c in range(NCORES)
    ]
    res = bass_utils.run_bass_kernel_spmd(
        nc, in_maps, core_ids=list(range(NCORES))
    )
    o = np.stack([r["out"] for r in res.results]).astype(np.float64)
    num = o[:, 0, 0].sum()
    den = float(np.asarray(mask, np.int64).sum() - B)
    return np.asarray(num / den, dtype=np.float32)


# revision 9
# speedup vs baseline: 1.1851x; 1.0911x over previous
"""Trainium2 Bass kernel for nn_DLI_loss_full.

Key algebraic fact: logits[b,j,k] = hw[b,j] + xw[b,k] and the loss is
sum(lse - tgt) over valid groups, so the hw[b,j] term (the whole LSTM
path) cancels exactly:

    per_group[b,j] = log(sum_{k=j+1}^{len_b-1} exp(xw[b,k])) - xw[b,j+1]
    loss = sum(per_group) / sum_b(len_b - 1)

with xw = encoder_output @ w_fc[HID:].  The kernel therefore only
streams encoder_output once (memory-bound), computes xw via
multiply+256-wide reductions, then gets every suffix log-sum-exp with
one hardware suffix-sum scan per 48-wide chunk plus a cross-chunk
combine done as a tiny 128x128 matmul.

Per-core layout: 16 batches x 8 chunks of 48 timesteps = 128 SBUF
partitions, each partition's encoder slice contiguous in DRAM.  The
encoder stream rides the scalar HWDGE ring alone (~340-420 B/ns
sustained); consts ride the sync ring.

Engine plan (v4, measured op costs): DVE is the conveyor - it runs
multiply (1.75us) or reduce (1.75us) back to back from the moment
piece 0 lands; gpsimd shadows it with the 4 middle multiplies (3.4us
each - its 2-input port-bound floor).  Three-engine concurrency (ACT
reduces) was tried and SLOWED everything ~45% via SBUF contention, so
ACT only does the exp/ln tail.  The mask arrives pre-cast fp32 and
pre-multiplied by the chunk mask from the host, so gpsimd runs nothing
but the 4 multiplies.  The final [128,2] result is collapsed to [2,1]
by a PE matmul against a ones column so the output DMA is 2
descriptors instead of 128.
"""

from contextlib import ExitStack

import numpy as np

import concourse.bacc as bacc
import concourse.mybir as mybir
import concourse.tile as tile
from concourse import bass_utils

B, T, D, HID = 128, 384, 256, 256
NCORES = 8
BS = B // NCORES            # 16 batches per core
CH = 8                      # chunks per sequence
L = T // CH                 # 48 timesteps per chunk
P = BS * CH                 # 128 partitions
NP = 8                      # DMA/compute pieces along the free axis
LP = L // NP                # 6 timesteps per piece
F32 = mybir.dt.float32
EPS = 1e-30                 # keeps ln() finite on fully-masked tails
C2W = 2 * L + P + 1         # mkf | wmh | um | ones

# gpsimd owns the 6 earliest pieces (its 3.4us/piece chain is the long
# pole, so it must start the moment piece 0 lands); DVE multiplies the
# late pieces 6,7 and reduces everything
GP_PIECES = (0, 1, 2, 3, 4, 5)

_cache = {}


def _build_nc():
    nc = bacc.Bacc(
        "TRN2", target_bir_lowering=False, debug=False, num_devices=NCORES
    )
    x = nc.dram_tensor("x", [BS, T, D], F32, kind="ExternalInput").ap()
    wt = nc.dram_tensor("wt", [P, D], F32, kind="ExternalInput").ap()
    c2 = nc.dram_tensor("c2", [P, C2W], F32, kind="ExternalInput").ap()
    out = nc.dram_tensor("out", [1, 1], F32, kind="ExternalOutput").ap()

    add = mybir.AluOpType.add
    mult = mybir.AluOpType.mult
    bypass = mybir.AluOpType.bypass
    AX = mybir.AxisListType.X
    ACT = mybir.ActivationFunctionType

    with tile.TileContext(nc) as tc, ExitStack() as ctx:
        sp = ctx.enter_context(tc.tile_pool(name="small", bufs=1))
        xp = ctx.enter_context(tc.tile_pool(name="xp", bufs=NP))
        rp = ctx.enter_context(tc.tile_pool(name="prod", bufs=NP))
        pp = ctx.enter_context(tc.tile_pool(name="psum", bufs=2, space="PSUM"))

        # x-piece loads first, all on the scalar HWDGE queue
        x_p = x.rearrange("b (c n l) d -> (b c) n (l d)", c=CH, n=NP)
        xts = []
        for i in range(NP):
            xt = xp.tile([P, LP * D], F32, tag="x")
            nc.scalar.dma_start(xt[:], x_p[:, i, :])
            xts.append(xt)

        # consts on the sync ring: wt alone first (it gates wrep), then
        # the rest packed into one block
        w_sb = sp.tile([P, D], F32)
        nc.sync.dma_start(w_sb[:], wt)
        c2_sb = sp.tile([P, C2W], F32)
        nc.sync.dma_start(c2_sb[:], c2)
        mf = c2_sb[:, 0:L]                      # fp32 mask (host-cast)
        wm = c2_sb[:, L:2 * L]                  # mask * chunk-mask (host)
        u_sb = c2_sb[:, 2 * L:2 * L + P]        # cross-chunk combine
        ones = c2_sb[:, 2 * L + P:2 * L + P + 1]

        # warm the Exp table while DMA streams (reads w, the earliest
        # const); the Ln table load is pinned after the real exp below
        warm = sp.tile([P, 1], F32)
        nc.scalar.activation(warm[:], w_sb[:, 0:1], ACT.Exp)

        # replicate w LP times on-chip so the multiplies read a plain
        # contiguous operand (0-stride broadcast halves DVE rate)
        wrep = sp.tile([P, LP * D], F32)
        nc.vector.tensor_copy(wrep[:, 0:D], w_sb[:])
        nc.vector.tensor_copy(wrep[:, D:2 * D], wrep[:, 0:D])
        nc.vector.tensor_copy(wrep[:, 2 * D:4 * D], wrep[:, 0:2 * D])
        nc.vector.tensor_copy(wrep[:, 4 * D:6 * D], wrep[:, 2 * D:4 * D])

        # products: gpsimd multiplies the middle pieces
        pts = [None] * NP
        for i in GP_PIECES:
            pts[i] = rp.tile([P, LP * D], F32, tag="prod", name=f"pt{i}")
            nc.gpsimd.tensor_tensor(pts[i][:], xts[i][:], wrep[:], mult)

        # xw[p, t] = sum_d x[p, t, d] * w[d]: DVE multiplies pieces
        # 0,1,6,7 and reduces everything, enqueued in expected
        # data-readiness order (engine queues run in-order)
        xw = sp.tile([P, L], F32)

        def _reduce(i):
            p3 = pts[i][:].rearrange("p (l d) -> p l d", d=D)
            nc.vector.tensor_reduce(
                xw[:, i * LP:(i + 1) * LP], p3, axis=AX, op=add
            )

        def _vmult(i):
            pts[i] = rp.tile([P, LP * D], F32, tag="prod", name=f"pt{i}")
            nc.vector.tensor_tensor(pts[i][:], xts[i][:], wrep[:], mult)

        _reduce(0)
        _reduce(1)
        _reduce(2)
        _reduce(3)
        _vmult(6)
        _reduce(4)
        _vmult(7)
        _reduce(5)
        _reduce(6)
        _reduce(7)

        # masked exp, chunk totals, cross-chunk exclusive suffix via matmul
        em = sp.tile([P, L], F32)
        nc.scalar.activation(em[:], xw[:], ACT.Exp)
        # reads em -> cannot be hoisted before the exp; triggers the Ln
        # table load here so it overlaps the DVE tail below
        lnwarm = sp.tile([P, 1], F32)
        nc.scalar.activation(lnwarm[:], em[:, 0:1], ACT.Ln)
        em2 = sp.tile([P, L], F32)
        nc.vector.tensor_mul(em2[:], em[:], mf)
        tot = sp.tile([P, 1], F32)
        nc.vector.tensor_reduce(tot[:], em2[:], axis=AX, op=add)
        aps = pp.tile([P, 1], F32, tag="mm")
        nc.tensor.matmul(aps[:], u_sb, tot[:], start=True, stop=True)
        a_sb = sp.tile([P, 1], F32)
        # + EPS seeds every suffix sum, keeping ln() finite on
        # fully-masked tails
        nc.vector.tensor_scalar_add(a_sb[:], aps[:], EPS)

        # within-chunk suffix sums, seeded with the later-chunk total
        ss = sp.tile([P, L], F32)
        nc.vector.tensor_tensor_scan(
            ss[:][:, ::-1], em2[:][:, ::-1], em2[:][:, ::-1],
            initial=a_sb[:], op0=add, op1=bypass,
        )
        lt = sp.tile([P, L], F32)
        nc.scalar.activation(lt[:], ss[:], ACT.Ln)

        # loss terms: per-partition sum of (ln(suffix) - xw) over valid
        # groups, and the valid-group count; then collapse across
        # partitions with a tiny matmul so the output is [2,1]
        diff = sp.tile([P, L], F32)
        nc.vector.tensor_sub(diff[:], lt[:], xw[:])
        res = sp.tile([P, 1], F32)
        nc.vector.scalar_tensor_tensor(
            out=diff[:], in0=diff[:], scalar=1.0, in1=wm,
            op0=bypass, op1=mult, accum_out=res[:, 0:1],
        )
        aps2 = pp.tile([1, 1], F32, tag="mm2")
        nc.tensor.matmul(aps2[:], res[:], ones, start=True, stop=True)
        res2 = sp.tile([1, 1], F32)
        nc.vector.tensor_copy(res2[:], aps2[:])
        nc.sync.dma_start(out, res2[:])

    nc.compile()
    return nc


def _host_consts():
    w_idx = np.arange(P)
    um = (
        (w_idx[:, None] // CH == w_idx[None, :] // CH)
        & (w_idx[:, None] % CH > w_idx[None, :] % CH)
    ).astype(np.float32)
    cm = np.ones((P, L), np.float32)
    cm[w_idx % CH == 0, 0] = 0.0
    return um, cm


def _core_c2(mask_core, um, cm):
    """Pack mkf | wmh | um | ones into one [P, C2W] block."""
    mkf = np.asarray(mask_core).reshape(P, L).astype(np.float32)
    c2 = np.empty((P, C2W), np.float32)
    c2[:, 0:L] = mkf
    c2[:, L:2 * L] = mkf * cm
    c2[:, 2 * L:2 * L + P] = um
    c2[:, 2 * L + P] = 1.0
    return c2


def kernel(**inputs) -> np.ndarray:
    enc = np.ascontiguousarray(np.asarray(inputs["encoder_output"], np.float32))
    mask = np.asarray(inputs["mask"])
    w_fc = np.asarray(inputs["w_fc"], np.float32)

    if "nc" not in _cache:
        _cache["nc"] = _build_nc()
    nc = _cache["nc"]

    wt = np.ascontiguousarray(np.broadcast_to(w_fc[HID:], (P, D)), np.float32)
    um, cm = _host_consts()
    in_maps = [
        {
            "x": enc[c * BS:(c + 1) * BS],
            "wt": wt,
            "c2": _core_c2(mask[c * BS:(c + 1) * BS], um, cm),
        }
        for c in range(NCORES)
    ]
    res = bass_utils.run_bass_kernel_spmd(
        nc, in_maps, core_ids=list(range(NCORES))
    )
    o = np.stack([r["out"] for r in res.results]).astype(np.float64)
    num = o[:, 0, 0].sum()
    den = float(np.asarray(mask, np.int64).sum() - B)
    return np.asarray(num / den, dtype=np.float32)


# revision 10
# speedup vs baseline: 1.1980x; 1.0108x over previous
"""Trainium2 Bass kernel for nn_DLI_loss_full.

Key algebraic fact: logits[b,j,k] = hw[b,j] + xw[b,k] and the loss is
sum(lse - tgt) over valid groups, so the hw[b,j] term (the whole LSTM
path) cancels exactly:

    per_group[b,j] = log(sum_{k=j+1}^{len_b-1} exp(xw[b,k])) - xw[b,j+1]
    loss = sum(per_group) / sum_b(len_b - 1)

with xw = encoder_output @ w_fc[HID:].  The kernel therefore only
streams encoder_output once (memory-bound), computes xw via
multiply+256-wide reductions, then gets every suffix log-sum-exp with
one hardware suffix-sum scan per 48-wide chunk plus a cross-chunk
combine done as a tiny 128x128 matmul.

Per-core layout: 16 batches x 8 chunks of 48 timesteps = 128 SBUF
partitions, each partition's encoder slice contiguous in DRAM.  The
encoder stream rides the scalar HWDGE ring alone (~340-420 B/ns
sustained); consts ride the sync ring.

Engine plan (v4, measured op costs): DVE is the conveyor - it runs
multiply (1.75us) or reduce (1.75us) back to back from the moment
piece 0 lands; gpsimd shadows it with the 4 middle multiplies (3.4us
each - its 2-input port-bound floor).  Three-engine concurrency (ACT
reduces) was tried and SLOWED everything ~45% via SBUF contention, so
ACT only does the exp/ln tail.  The mask arrives pre-cast fp32 and
pre-multiplied by the chunk mask from the host, so gpsimd runs nothing
but the 4 multiplies.  The final [128,2] result is collapsed to [2,1]
by a PE matmul against a ones column so the output DMA is 2
descriptors instead of 128.
"""

from contextlib import ExitStack

import numpy as np

import concourse.bacc as bacc
import concourse.mybir as mybir
import concourse.tile as tile
from concourse import bass_utils

B, T, D, HID = 128, 384, 256, 256
NCORES = 8
BS = B // NCORES            # 16 batches per core
CH = 8                      # chunks per sequence
L = T // CH                 # 48 timesteps per chunk
P = BS * CH                 # 128 partitions
NP = 8                      # DMA/compute pieces along the free axis
LP = L // NP                # 6 timesteps per piece
F32 = mybir.dt.float32
EPS = 1e-30                 # keeps ln() finite on fully-masked tails
C2W = 2 * L + P + 1         # mkf | wmh | um | ones

# gpsimd owns the 6 earliest pieces (its 3.4us/piece chain is the long
# pole, so it must start the moment piece 0 lands); DVE multiplies the
# late pieces 6,7 and reduces everything
GP_PIECES = (0, 1, 2, 3, 4, 5)

_cache = {}


def _build_nc():
    nc = bacc.Bacc(
        "TRN2", target_bir_lowering=False, debug=False, num_devices=NCORES
    )
    x = nc.dram_tensor("x", [BS, T, D], F32, kind="ExternalInput").ap()
    wt = nc.dram_tensor("wt", [P, D], F32, kind="ExternalInput").ap()
    c2 = nc.dram_tensor("c2", [P, C2W], F32, kind="ExternalInput").ap()
    out = nc.dram_tensor("out", [1, 1], F32, kind="ExternalOutput").ap()

    add = mybir.AluOpType.add
    mult = mybir.AluOpType.mult
    bypass = mybir.AluOpType.bypass
    AX = mybir.AxisListType.X
    ACT = mybir.ActivationFunctionType

    with tile.TileContext(nc) as tc, ExitStack() as ctx:
        sp = ctx.enter_context(tc.tile_pool(name="small", bufs=1))
        xp = ctx.enter_context(tc.tile_pool(name="xp", bufs=NP))
        rp = ctx.enter_context(tc.tile_pool(name="prod", bufs=NP))
        pp = ctx.enter_context(tc.tile_pool(name="psum", bufs=2, space="PSUM"))

        # x-piece loads first, all on the scalar HWDGE queue
        x_p = x.rearrange("b (c n l) d -> (b c) n (l d)", c=CH, n=NP)
        xts = []
        for i in range(NP):
            xt = xp.tile([P, LP * D], F32, tag="x")
            # pieces 6,7 ride the sync ring: round-robin across the two
            # HWDGE rings lands them ~10us earlier than queue position 7
            # on the scalar ring, so the DVE multiplies never data-stall
            eng = nc.scalar if i < 6 else nc.sync
            eng.dma_start(xt[:], x_p[:, i, :])
            xts.append(xt)

        # consts on the sync ring: wt alone first (it gates wrep), then
        # the rest packed into one block
        w_sb = sp.tile([P, D], F32)
        nc.sync.dma_start(w_sb[:], wt)
        c2_sb = sp.tile([P, C2W], F32)
        nc.sync.dma_start(c2_sb[:], c2)
        mf = c2_sb[:, 0:L]                      # fp32 mask (host-cast)
        wm = c2_sb[:, L:2 * L]                  # mask * chunk-mask (host)
        u_sb = c2_sb[:, 2 * L:2 * L + P]        # cross-chunk combine
        ones = c2_sb[:, 2 * L + P:2 * L + P + 1]

        # warm the Exp table while DMA streams (reads w, the earliest
        # const); the Ln table load is pinned after the real exp below
        warm = sp.tile([P, 1], F32)
        nc.scalar.activation(warm[:], w_sb[:, 0:1], ACT.Exp)

        # replicate w LP times on-chip so the multiplies read a plain
        # contiguous operand (0-stride broadcast halves DVE rate)
        wrep = sp.tile([P, LP * D], F32)
        nc.vector.tensor_copy(wrep[:, 0:D], w_sb[:])
        nc.vector.tensor_copy(wrep[:, D:2 * D], wrep[:, 0:D])
        nc.vector.tensor_copy(wrep[:, 2 * D:4 * D], wrep[:, 0:2 * D])
        nc.vector.tensor_copy(wrep[:, 4 * D:6 * D], wrep[:, 2 * D:4 * D])

        # products: gpsimd multiplies the middle pieces
        pts = [None] * NP
        for i in GP_PIECES:
            pts[i] = rp.tile([P, LP * D], F32, tag="prod", name=f"pt{i}")
            nc.gpsimd.tensor_tensor(pts[i][:], xts[i][:], wrep[:], mult)

        # xw[p, t] = sum_d x[p, t, d] * w[d]: DVE multiplies pieces
        # 0,1,6,7 and reduces everything, enqueued in expected
        # data-readiness order (engine queues run in-order)
        xw = sp.tile([P, L], F32)

        def _reduce(i):
            p3 = pts[i][:].rearrange("p (l d) -> p l d", d=D)
            nc.vector.tensor_reduce(
                xw[:, i * LP:(i + 1) * LP], p3, axis=AX, op=add
            )

        def _vmult(i):
            pts[i] = rp.tile([P, LP * D], F32, tag="prod", name=f"pt{i}")
            nc.vector.tensor_tensor(pts[i][:], xts[i][:], wrep[:], mult)

        _reduce(0)
        _reduce(1)
        _reduce(2)
        _vmult(6)
        _reduce(3)
        _vmult(7)
        _reduce(4)
        _reduce(6)
        _reduce(7)
        _reduce(5)   # gated on gpsimd's last mult - keep it last

        # masked exp, chunk totals, cross-chunk exclusive suffix via matmul
        em = sp.tile([P, L], F32)
        nc.scalar.activation(em[:], xw[:], ACT.Exp)
        # reads em -> cannot be hoisted before the exp; triggers the Ln
        # table load here so it overlaps the DVE tail below
        lnwarm = sp.tile([P, 1], F32)
        nc.scalar.activation(lnwarm[:], em[:, 0:1], ACT.Ln)
        em2 = sp.tile([P, L], F32)
        nc.vector.tensor_mul(em2[:], em[:], mf)
        tot = sp.tile([P, 1], F32)
        nc.vector.tensor_reduce(tot[:], em2[:], axis=AX, op=add)
        aps = pp.tile([P, 1], F32, tag="mm")
        nc.tensor.matmul(aps[:], u_sb, tot[:], start=True, stop=True)
        a_sb = sp.tile([P, 1], F32)
        # + EPS seeds every suffix sum, keeping ln() finite on
        # fully-masked tails
        nc.vector.tensor_scalar_add(a_sb[:], aps[:], EPS)

        # within-chunk suffix sums, seeded with the later-chunk total
        ss = sp.tile([P, L], F32)
        nc.vector.tensor_tensor_scan(
            ss[:][:, ::-1], em2[:][:, ::-1], em2[:][:, ::-1],
            initial=a_sb[:], op0=add, op1=bypass,
        )
        lt = sp.tile([P, L], F32)
        nc.scalar.activation(lt[:], ss[:], ACT.Ln)

        # loss terms: per-partition sum of (ln(suffix) - xw) over valid
        # groups, and the valid-group count; then collapse across
        # partitions with a tiny matmul so the output is [2,1]
        diff = sp.tile([P, L], F32)
        nc.vector.tensor_sub(diff[:], lt[:], xw[:])
        res = sp.tile([P, 1], F32)
        nc.vector.scalar_tensor_tensor(
            out=diff[:], in0=diff[:], scalar=1.0, in1=wm,
            op0=bypass, op1=mult, accum_out=res[:, 0:1],
        )
        aps2 = pp.tile([1, 1], F32, tag="mm2")
        nc.tensor.matmul(aps2[:], res[:], ones, start=True, stop=True)
        res2 = sp.tile([1, 1], F32)
        nc.vector.tensor_copy(res2[:], aps2[:])
        nc.sync.dma_start(out, res2[:])

    nc.compile()
    return nc


def _host_consts():
    w_idx = np.arange(P)
    um = (
        (w_idx[:, None] // CH == w_idx[None, :] // CH)
        & (w_idx[:, None] % CH > w_idx[None, :] % CH)
    ).astype(np.float32)
    cm = np.ones((P, L), np.float32)
    cm[w_idx % CH == 0, 0] = 0.0
    return um, cm


def _core_c2(mask_core, um, cm):
    """Pack mkf | wmh | um | ones into one [P, C2W] block."""
    mkf = np.asarray(mask_core).reshape(P, L).astype(np.float32)
    c2 = np.empty((P, C2W), np.float32)
    c2[:, 0:L] = mkf
    c2[:, L:2 * L] = mkf * cm
    c2[:, 2 * L:2 * L + P] = um
    c2[:, 2 * L + P] = 1.0
    return c2


def kernel(**inputs) -> np.ndarray:
    enc = np.ascontiguousarray(np.asarray(inputs["encoder_output"], np.float32))
    mask = np.asarray(inputs["mask"])
    w_fc = np.asarray(inputs["w_fc"], np.float32)

    if "nc" not in _cache:
        _cache["nc"] = _build_nc()
    nc = _cache["nc"]

    wt = np.ascontiguousarray(np.broadcast_to(w_fc[HID:], (P, D)), np.float32)
    um, cm = _host_consts()
    in_maps = [
        {
            "x": enc[c * BS:(c + 1) * BS],
            "wt": wt,
            "c2": _core_c2(mask[c * BS:(c + 1) * BS], um, cm),
        }
        for c in range(NCORES)
    ]
    res = bass_utils.run_bass_kernel_spmd(
        nc, in_maps, core_ids=list(range(NCORES))
    )
    o = np.stack([r["out"] for r in res.results]).astype(np.float64)
    num = o[:, 0, 0].sum()
    den = float(np.asarray(mask, np.int64).sum() - B)
    return np.asarray(num / den, dtype=np.float32)
